# revision 16
# baseline (speedup 1.0000x reference)
"""Bass/Tile TRN2 kernel for nn_Attention_5428838662814.

Math (per batch b):
    enc = out_e[:, b, :256] + out_e[:, b, 256:]        # [S, H]
    scores[s, t] = sum_h enc[s, h] * dec[t, h]          # [S, T]
    P = softmax(scores, axis=s)
    out[t, h] = sum_s P[s, t] * enc[s, h]               # [T, H]

Kernel strategy:
  - Data-parallel over batch: B=16 across 8 cores, 2 batches/core.
  - scores in [s, t] layout so U = exp(scores - C) is directly the
    stationary operand of the AV matmul; rhs = [enc | ones] gives the
    context numerator and the softmax denominator in one pass.
  - Fixed shift C=90 replaces the per-column max (scores ~ N(0, 512)).
  - The PE runs instructions in order and its clock ramps only under
    continuous execution (0.65 -> 1.2 -> 2.4 GHz after ~3us busy), so the
    DMA-paced head is kept DENSE: batch 0's stage-1 transposes + its first
    QK phase are fused and padded with dummy matmuls sized to the DMA
    arrival pace. Engine choreography in the head: e-sum adds on Pool,
    ench copies on ACT, transpose PSUM->SBUF copies on DVE.
  - 9 phases of (batch, t-block): 16x2 QK matmuls + exps each, hosting the
    PREVIOUS phase's AV groups (one per 4 iters). Batch 1's stage-1 is
    fused into its own first phase (data lands ~40us earlier). The last
    phase is split 2x256 so only 2 AV groups (~3.6us) drain at the end.
  - QK precision: f32r (tf32-like) single pass, rel err ~4.3e-3.
    AV pass: U and enc in bf16 (U needs fp32-range exponent).
"""

import os
from collections import deque

import numpy as np

import concourse.bass as bass
import concourse.bacc as bacc
import concourse.mybir as mybir
import concourse.tile as tile
from concourse import bass_utils
from concourse.masks import make_identity

S = 2048          # source positions
T = 2048          # target positions
H = 256           # head dim
B = 16            # global batch
N_CORES = 8
BL = B // N_CORES  # batches per core
P = 128
C_SHIFT = 90.0
NT_S = S // P      # 16 s-tiles
TBLK = 512         # t-block width for QK scores
NBLK = T // TBLK   # 4
KK = H // P        # 2 contraction k-tiles

bf = mybir.dt.bfloat16
f16 = mybir.dt.float16
f32 = mybir.dt.float32
f32r = mybir.dt.float32r
EXP = mybir.ActivationFunctionType.Exp

QK_MODE = os.environ.get("ATTN_QK_MODE", "f32r")
WARMUP = int(os.environ.get("ATTN_WARMUP", "12"))
WFILL = int(os.environ.get("ATTN_WFILL", "3"))


def build_program():
    nc = bacc.Bacc("TRN2", target_bir_lowering=False, debug=False)
    e = nc.dram_tensor("e", [S, BL, 2 * H], f32, kind="ExternalInput").ap()
    d = nc.dram_tensor("d", [T, BL, H], f32, kind="ExternalInput").ap()
    o = nc.dram_tensor("o", [T, BL, H], f32, kind="ExternalOutput").ap()

    tdt = f32r if QK_MODE == "f32r" else f16
    xdt = f32 if QK_MODE == "f32r" else f16

    with tile.TileContext(nc) as tc:
        with (
            tc.tile_pool(name="const", bufs=1) as constp,
            tc.tile_pool(name="stage", bufs=4) as stage,
            tc.tile_pool(name="persist", bufs=1) as persist,
            tc.tile_pool(name="ubp", bufs=2) as ubp,
            tc.tile_pool(name="outp", bufs=4) as outp,
            tc.tile_pool(name="qkps", bufs=4, space="PSUM") as qkps,
            tc.tile_pool(name="qkps2", bufs=2, space="PSUM") as qkps2,
            tc.tile_pool(name="avps", bufs=2, space="PSUM") as avps,
        ):
            ident = constp.tile([P, P], bf)
            make_identity(nc, ident)
            identf = constp.tile([P, P], xdt, tag="identf")
            make_identity(nc, identf)
            cbias = constp.tile([P, 1], f32, tag="cbias")
            nc.vector.memset(cbias[:, :], -C_SHIFT)

            nwarm = [0]

            def warm(n):
                """Dummy matmuls: keep the PE dense / p-state ramping while
                real head work is DMA-paced."""
                for _ in range(n):
                    w = qkps2.tile([P, P], f32, tag="qk2",
                                   name=f"w{nwarm[0]}")
                    nwarm[0] += 1
                    nc.tensor.matmul(w[:, :], ident[:, :], ident[:, :],
                                     start=True, stop=True)

            warm(WARMUP)
            wact = constp.tile([P, 1], f32, tag="wact")
            nc.scalar.activation(wact[:, :], cbias[:, :], EXP,
                                 bias=cbias[:, :], scale=1.0)

            # ---- persistent per-batch buffers ----
            ench = {}
            encT = {}
            decT = {}
            for b in range(BL):
                ench[b] = persist.tile([P, NT_S, H + 4], bf, tag=f"ench{b}",
                                       name=f"ench{b}")
                encT[b] = persist.tile([P, KK, S], tdt, tag=f"encT{b}",
                                       name=f"encT{b}")
                decT[b] = persist.tile([P, KK, T], tdt, tag=f"decT{b}",
                                       name=f"decT{b}")
                nc.vector.memset(ench[b][:, :, H:H + 1], 1.0)

            # per-s-tile landing buffers (enc summed on Pool; dec raw)
            e32 = {}
            dfb = {}
            for b in range(BL):
                for i in range(NT_S):
                    e32[b, i] = persist.tile([P, H], f32, tag=f"e32_{b}_{i}",
                                             name=f"e32_{b}_{i}")
                    dfb[b, i] = persist.tile([P, H], f32, tag=f"df_{b}_{i}",
                                             name=f"df_{b}_{i}")

            def load_tile(b, i):
                """DMA s-tile i of batch b; enc halves summed on Pool."""
                rows = slice(i * P, (i + 1) * P)
                ef = stage.tile([P, 2 * H], f32, tag="ef",
                                name=f"ef{b}_{i}")
                nc.sync.dma_start(ef[:, :], e[rows, b, :])
                nc.gpsimd.tensor_add(e32[b, i][:, :], ef[:, 0:H],
                                     ef[:, H:2 * H])
                nc.sync.dma_start(dfb[b, i][:, :], d[rows, b, :])

            def transpose_pair(src, dst, i):
                """PE-transpose [P, H] src into dst[:, kk, i*P:(i+1)*P]."""
                if xdt != f32:
                    s16 = stage.tile([P, H], xdt, tag="s16",
                                     name=f"s16_{dst.name}_{i}")
                    nc.vector.tensor_copy(s16[:, :], src[:, :])
                    src = s16
                for kk in range(KK):
                    pt = qkps2.tile([P, P], xdt, tag="qk2",
                                    name=f"tp_{dst.name}_{i}_{kk}")
                    nc.tensor.transpose(pt[:, :],
                                        src[:, kk * P:(kk + 1) * P],
                                        identf[:, :])
                    nc.vector.tensor_copy(dst[:, kk, i * P:(i + 1) * P],
                                          pt[:, :])

            def av_group(bv, t0, ub_j, tt):
                """One output tile [P, H]: AV matmuls + normalize + store."""
                av = avps.tile([P, H + 1], f32, tag="av",
                               name=f"av{bv}_{t0}")
                for i in range(NT_S):
                    nc.tensor.matmul(
                        av[:, :],
                        ub_j[:, i, tt * P:(tt + 1) * P],
                        ench[bv][:, i, 0:H + 1],
                        start=(i == 0),
                        stop=(i == NT_S - 1),
                    )
                den = outp.tile([P, 1], f32, tag="den", name=f"dn{bv}_{t0}")
                nc.vector.reciprocal(den[:, :], av[:, H:H + 1])
                ot = outp.tile([P, H], f32, tag="ot", name=f"ot{bv}_{t0}")
                nc.vector.tensor_scalar_mul(ot[:, :], av[:, 0:H], den[:, :])
                nc.sync.dma_start(o[t0:t0 + P, bv, :], ot[:, :])

            # ---- phase list: (b, t0, width, fused) ----
            phases = []
            for b in range(BL):
                for j in range(NBLK):
                    if b == BL - 1 and j == NBLK - 1:
                        h2 = TBLK // 2
                        phases.append((b, j * TBLK, h2, False))
                        phases.append((b, j * TBLK + h2, h2, False))
                    else:
                        phases.append((b, j * TBLK, TBLK, j == 0))

            # batch-0 s-tiles 0..3 first: their d-columns gate phase j0
            for i in range(4):
                load_tile(0, i)

            pending = deque()  # AV groups ready to host: (b, t0, ub, tt)
            phase_no = 0
            for (b, t0, w, fused) in phases:
                if fused:
                    for i in range(4):
                        transpose_pair(dfb[b, i], decT[b], i)
                ub = ubp.tile([P, NT_S, w], bf, tag="ub",
                              name=f"ub{b}_{t0}")
                for i in range(NT_S):
                    if fused:
                        # stage-1 for s-tile i: ench copy on ACT (b1's is
                        # done on DVE in phase 2), transposes on PE
                        if b == 0:
                            nc.scalar.copy(ench[b][:, i, 0:H],
                                           e32[b, i][:, :])
                        transpose_pair(e32[b, i], encT[b], i)
                        if i < NT_S - 4:
                            if b == 0:
                                load_tile(0, i + 4)
                            transpose_pair(dfb[b, i + 4], decT[b], i + 4)
                    ps = (qkps if w == TBLK else qkps2).tile(
                        [P, w], f32, tag="qk" if w == TBLK else "qk2",
                        name=f"qk{b}_{t0}_{i}")
                    for kk in range(KK):
                        nc.tensor.matmul(
                            ps[:, :],
                            encT[b][:, kk, i * P:(i + 1) * P],
                            decT[b][:, kk, t0:t0 + w],
                            start=(kk == 0),
                            stop=(kk == KK - 1),
                        )
                    nc.scalar.activation(
                        ub[:, i, :], ps[:, :], EXP,
                        bias=cbias[:, :], scale=1.0,
                    )
                    if fused and b == 0:
                        # pad the DMA-paced head so the PE never idles
                        warm(WFILL)
                    if phase_no == 2:
                        # batch-1 ench copies, spread on DVE (ACT is
                        # exp-bound in-phase, DVE has slack here)
                        nc.vector.tensor_copy(ench[1][:, i, 0:H],
                                              e32[1, i][:, :])
                    if i % 4 == 3 and pending:
                        av_group(*pending.popleft())
                if b == 0 and fused:
                    # batch-1 input DMAs: stream behind batch 0's on the
                    # rings; Pool does the adds; transposes happen in
                    # batch 1's own fused phase (~40us after landing)
                    for i in range(NT_S):
                        load_tile(1, i)
                for tt in range(w // P):
                    pending.append((b, t0 + tt * P, ub, tt))
                phase_no += 1

            while pending:
                av_group(*pending.popleft())

    nc.compile()
    return nc


_NC_CACHE = []


def _get_nc():
    if not _NC_CACHE:
        _NC_CACHE.append(build_program())
    return _NC_CACHE[0]


def kernel(out_e, out_d, _trace=False, _trace_kwargs=None):
    assert out_e.shape == (S, B, 2 * H) and out_d.shape == (T, B, H)
    nc = _get_nc()
    in_maps = []
    for c in range(N_CORES):
        bs = slice(c * BL, (c + 1) * BL)
        in_maps.append({
            "e": np.ascontiguousarray(out_e[:, bs, :], dtype=np.float32),
            "d": np.ascontiguousarray(out_d[:, bs, :], dtype=np.float32),
        })
    res = bass_utils.run_bass_kernel_spmd(
        nc, in_maps, core_ids=list(range(N_CORES)),
        trace=_trace, **(_trace_kwargs or {}),
    )
    out = np.concatenate([res.results[c]["o"] for c in range(N_CORES)], axis=1)
    if _trace:
        return out.astype(np.float32), res
    return out.astype(np.float32)


# revision 19
# speedup vs baseline: 1.2220x; 1.2220x over previous
"""Bass/Tile TRN2 kernel for nn_Attention_5428838662814.

Math (per batch b):
    enc = out_e[:, b, :256] + out_e[:, b, 256:]        # [S, H]
    scores[s, t] = sum_h enc[s, h] * dec[t, h]          # [S, T]
    P = softmax(scores, axis=s)
    out[t, h] = sum_s P[s, t] * enc[s, h]               # [T, H]

Kernel strategy:
  - Data-parallel over batch: B=16 across 8 cores, 2 batches/core.
  - scores in [s, t] layout so U = exp(scores - C) is directly the
    stationary operand of the AV matmul; rhs = [enc | ones] gives the
    context numerator and the softmax denominator in one pass.
  - Fixed shift C=90 replaces the per-column max (scores ~ N(0, 512)).
  - The PE runs instructions in order and its clock ramps only under
    continuous execution (0.65 -> 1.2 -> 2.4 GHz after ~3us busy), so the
    DMA-paced head is kept DENSE: batch 0's stage-1 transposes + its first
    QK phase are fused and padded with dummy matmuls sized to the DMA
    arrival pace. Engine choreography in the head: e-sum adds on Pool,
    ench copies on ACT, transpose PSUM->SBUF copies on DVE.
  - 9 phases of (batch, t-block): 16x2 QK matmuls + exps each, hosting the
    PREVIOUS phase's AV groups (one per 4 iters). Batch 1's stage-1 is
    fused into its own first phase (data lands ~40us earlier). The last
    phase is split 2x256 so only 2 AV groups (~3.6us) drain at the end.
  - QK precision: f32r (tf32-like) single pass, rel err ~4.3e-3.
    AV pass: U and enc in bf16 (U needs fp32-range exponent).
"""

import os
from collections import deque

import numpy as np

import concourse.bass as bass
import concourse.bacc as bacc
import concourse.mybir as mybir
import concourse.tile as tile
from concourse import bass_utils
from concourse.masks import make_identity

S = 2048          # source positions
T = 2048          # target positions
H = 256           # head dim
B = 16            # global batch
N_CORES = 8
BL = B // N_CORES  # batches per core
P = 128
C_SHIFT = 90.0
NT_S = S // P      # 16 s-tiles
TBLK = 512         # t-block width for QK scores
NBLK = T // TBLK   # 4
KK = H // P        # 2 contraction k-tiles

bf = mybir.dt.bfloat16
f16 = mybir.dt.float16
f32 = mybir.dt.float32
f32r = mybir.dt.float32r
EXP = mybir.ActivationFunctionType.Exp

QK_MODE = os.environ.get("ATTN_QK_MODE", "f32r")
WARMUP = int(os.environ.get("ATTN_WARMUP", "6"))
WFILL = int(os.environ.get("ATTN_WFILL", "3"))


def build_program():
    nc = bacc.Bacc("TRN2", target_bir_lowering=False, debug=False)
    e = nc.dram_tensor("e", [S, BL, 2 * H], f32, kind="ExternalInput").ap()
    d = nc.dram_tensor("d", [T, BL, H], f32, kind="ExternalInput").ap()
    o = nc.dram_tensor("o", [T, BL, H], f32, kind="ExternalOutput").ap()

    tdt = f32r if QK_MODE == "f32r" else f16
    xdt = f32 if QK_MODE == "f32r" else f16

    with tile.TileContext(nc) as tc:
        with (
            tc.tile_pool(name="const", bufs=1) as constp,
            tc.tile_pool(name="stage", bufs=4) as stage,
            tc.tile_pool(name="persist", bufs=1) as persist,
            tc.tile_pool(name="ubp", bufs=2) as ubp,
            tc.tile_pool(name="outp", bufs=4) as outp,
            tc.tile_pool(name="qkps", bufs=3, space="PSUM") as qkps,
            tc.tile_pool(name="qkps2", bufs=3, space="PSUM") as qkps2,
            tc.tile_pool(name="avps", bufs=2, space="PSUM") as avps,
        ):
            ident = constp.tile([P, P], bf)
            make_identity(nc, ident)
            identf = constp.tile([P, P], xdt, tag="identf")
            make_identity(nc, identf)
            cbias = constp.tile([P, 1], f32, tag="cbias")
            nc.vector.memset(cbias[:, :], -C_SHIFT)

            nwarm = [0]

            def warm(n):
                """Dummy matmuls: keep the PE dense / p-state ramping while
                real head work is DMA-paced."""
                for _ in range(n):
                    w = qkps2.tile([P, P], f32, tag="qk2",
                                   name=f"w{nwarm[0]}")
                    nwarm[0] += 1
                    nc.tensor.matmul(w[:, :], ident[:, :], ident[:, :],
                                     start=True, stop=True)

            warm(WARMUP)
            wact = constp.tile([P, 1], f32, tag="wact")
            nc.scalar.activation(wact[:, :], cbias[:, :], EXP,
                                 bias=cbias[:, :], scale=1.0)

            # ---- persistent per-batch buffers ----
            ench = {}
            encT = {}
            decT = {}
            for b in range(BL):
                ench[b] = persist.tile([P, NT_S, H + 4], bf, tag=f"ench{b}",
                                       name=f"ench{b}")
                encT[b] = persist.tile([P, KK, S], tdt, tag=f"encT{b}",
                                       name=f"encT{b}")
                decT[b] = persist.tile([P, KK, T], tdt, tag=f"decT{b}",
                                       name=f"decT{b}")
                nc.vector.memset(ench[b][:, :, H:H + 1], 1.0)

            # per-s-tile landing buffers (enc summed on Pool; dec raw)
            e32 = {}
            dfb = {}
            for b in range(BL):
                for i in range(NT_S):
                    e32[b, i] = persist.tile([P, H], f32, tag=f"e32_{b}_{i}",
                                             name=f"e32_{b}_{i}")
                    dfb[b, i] = persist.tile([P, H], f32, tag=f"df_{b}_{i}",
                                             name=f"df_{b}_{i}")

            def load_tile(b, i):
                """DMA s-tile i of batch b; enc halves summed on Pool."""
                rows = slice(i * P, (i + 1) * P)
                ef = stage.tile([P, 2 * H], f32, tag="ef",
                                name=f"ef{b}_{i}")
                nc.sync.dma_start(ef[:, :], e[rows, b, :])
                nc.vector.tensor_add(e32[b, i][:, :], ef[:, 0:H],
                                     ef[:, H:2 * H])
                nc.sync.dma_start(dfb[b, i][:, :], d[rows, b, :])

            def transpose_pair(src, dst, i):
                """PE-transpose [P, H] src into dst[:, kk, i*P:(i+1)*P]."""
                if xdt != f32:
                    s16 = stage.tile([P, H], xdt, tag="s16",
                                     name=f"s16_{dst.name}_{i}")
                    nc.vector.tensor_copy(s16[:, :], src[:, :])
                    src = s16
                for kk in range(KK):
                    pt = qkps2.tile([P, P], xdt, tag="qk2",
                                    name=f"tp_{dst.name}_{i}_{kk}")
                    nc.tensor.transpose(pt[:, :],
                                        src[:, kk * P:(kk + 1) * P],
                                        identf[:, :])
                    nc.vector.tensor_copy(dst[:, kk, i * P:(i + 1) * P],
                                          pt[:, :])

            def av_group(bv, t0, ub_j, tt):
                """One output tile [P, H]: AV matmuls + normalize + store."""
                av = avps.tile([P, H + 1], f32, tag="av",
                               name=f"av{bv}_{t0}")
                for i in range(NT_S):
                    nc.tensor.matmul(
                        av[:, :],
                        ub_j[:, i, tt * P:(tt + 1) * P],
                        ench[bv][:, i, 0:H + 1],
                        start=(i == 0),
                        stop=(i == NT_S - 1),
                    )
                den = outp.tile([P, 1], f32, tag="den", name=f"dn{bv}_{t0}")
                nc.vector.reciprocal(den[:, :], av[:, H:H + 1])
                ot = outp.tile([P, H], f32, tag="ot", name=f"ot{bv}_{t0}")
                nc.vector.tensor_scalar_mul(ot[:, :], av[:, 0:H], den[:, :])
                nc.sync.dma_start(o[t0:t0 + P, bv, :], ot[:, :])

            # ---- phase list: (b, t0, width, fused) ----
            phases = []
            for b in range(BL):
                for j in range(NBLK):
                    if b == BL - 1 and j == NBLK - 1:
                        h2 = TBLK // 2
                        phases.append((b, j * TBLK, h2, False))
                        phases.append((b, j * TBLK + h2, h2, False))
                    else:
                        phases.append((b, j * TBLK, TBLK, j == 0))

            # batch-0 s-tiles 0..3 first: their d-columns gate phase j0
            for i in range(4):
                load_tile(0, i)

            pending = deque()  # AV groups ready to host: (b, t0, ub, tt)
            phase_no = 0
            for (b, t0, w, fused) in phases:
                if fused:
                    for i in range(4):
                        transpose_pair(dfb[b, i], decT[b], i)
                ub = ubp.tile([P, NT_S, w], bf, tag="ub",
                              name=f"ub{b}_{t0}")
                for i in range(NT_S):
                    if fused:
                        # stage-1 for s-tile i: ench copy on ACT (b1's is
                        # done on DVE in phase 2), transposes on PE
                        if b == 0:
                            nc.scalar.copy(ench[b][:, i, 0:H],
                                           e32[b, i][:, :])
                        transpose_pair(e32[b, i], encT[b], i)
                        if i < NT_S - 4:
                            if b == 0:
                                load_tile(0, i + 4)
                            transpose_pair(dfb[b, i + 4], decT[b], i + 4)
                    ps = (qkps if w == TBLK else qkps2).tile(
                        [P, w], f32, tag="qk" if w == TBLK else "qk2",
                        name=f"qk{b}_{t0}_{i}")
                    for kk in range(KK):
                        nc.tensor.matmul(
                            ps[:, :],
                            encT[b][:, kk, i * P:(i + 1) * P],
                            decT[b][:, kk, t0:t0 + w],
                            start=(kk == 0),
                            stop=(kk == KK - 1),
                        )
                    nc.scalar.activation(
                        ub[:, i, :], ps[:, :], EXP,
                        bias=cbias[:, :], scale=1.0,
                    )
                    if fused and b == 0:
                        # pad the DMA-paced head so the PE never idles
                        warm(WFILL)
                    if phase_no == 2:
                        # batch-1 ench copies, spread on DVE (ACT is
                        # exp-bound in-phase, DVE has slack here)
                        nc.vector.tensor_copy(ench[1][:, i, 0:H],
                                              e32[1, i][:, :])
                    if i % 4 == 3 and pending:
                        av_group(*pending.popleft())
                if b == 0 and fused:
                    # batch-1 input DMAs: stream behind batch 0's on the
                    # rings; Pool does the adds; transposes happen in
                    # batch 1's own fused phase (~40us after landing)
                    for i in range(NT_S):
                        load_tile(1, i)
                for tt in range(w // P):
                    pending.append((b, t0 + tt * P, ub, tt))
                phase_no += 1

            while pending:
                av_group(*pending.popleft())

    nc.compile()
    return nc


_NC_CACHE = []


def _get_nc():
    if not _NC_CACHE:
        _NC_CACHE.append(build_program())
    return _NC_CACHE[0]


def kernel(out_e, out_d, _trace=False, _trace_kwargs=None):
    assert out_e.shape == (S, B, 2 * H) and out_d.shape == (T, B, H)
    nc = _get_nc()
    in_maps = []
    for c in range(N_CORES):
        bs = slice(c * BL, (c + 1) * BL)
        in_maps.append({
            "e": np.ascontiguousarray(out_e[:, bs, :], dtype=np.float32),
            "d": np.ascontiguousarray(out_d[:, bs, :], dtype=np.float32),
        })
    res = bass_utils.run_bass_kernel_spmd(
        nc, in_maps, core_ids=list(range(N_CORES)),
        trace=_trace, **(_trace_kwargs or {}),
    )
    out = np.concatenate([res.results[c]["o"] for c in range(N_CORES)], axis=1)
    if _trace:
        return out.astype(np.float32), res
    return out.astype(np.float32)
